# revision 1
# baseline (speedup 1.0000x reference)
"""TAGConv×2 GNN discriminator on 8 Trainium2 NeuronCores.

Strategy (per sharding hint): shard destination nodes across 8 cores
(6272 padded rows each), replicate per-hop weights, exchange the
propagated node-feature table between hops via AllGather, all-reduce the
global pool result.

Device algorithm per A-application ("hop"):
  - dma_gather pulls 128-edge chunks of source-node feature rows (bf16,
    256B rows) from the replicated DRAM table (split in two halves to fit
    int16 gather indices).
  - A selection matrix S[e, r] = norm[e] * (rowloc[e] == r), built on the
    vector engine from compact per-edge metadata, scatter-adds each chunk
    into a PSUM block via a TensorE matmul (out[128 rows, 128 feat] +=
    S^T @ gathered).
  - The dense Horner term x@W[k] (or h1@W2[k]) accumulates into the same
    PSUM tile, so out = x@W[k] + A·t comes out of PSUM directly.
Layer ends apply bias + PReLU; layer 2's final hop feeds a masked pool
matmul; the pooled vector hits Wout and an AllReduce finishes the job.
"""
import hashlib
import math
import os

import numpy as np
import ml_dtypes

BF = ml_dtypes.bfloat16

NCORES = 8
NNODES = 50000
FEAT = 128
KHOPS = 3
NBLK = 49                 # 128-row blocks per core
RPC = NBLK * 128          # rows per core (6272)
NPAD = NCORES * RPC       # padded node count (50176)
SPLIT = 32768             # int16 gather-index split point
GIDX = 1024               # indices per dma_gather call
SGRP = 32                 # subchunks per S-build DVE op

_CACHE: dict = {}

# dma_gather cannot read internal (scratchpad-relative) DRAM — tables must be
# I/O tensors. If True, AllGather writes the ExternalOutput table directly;
# if False, AllGather lands in an internal Shared bounce and a dram2dram copy
# moves it into the ExternalOutput table.
DIRECT_CC_TABLES = False
# Debug: emit only the first K_STEPS stages (1..8); 8 = full program.
K_STEPS = int(os.environ.get("K_STEPS", "8"))


# ----------------------------------------------------------------------------
# Host preprocessing: graph -> per-core gather streams + program structure
# ----------------------------------------------------------------------------

def _preprocess(edge_index: np.ndarray):
    row = edge_index[0].astype(np.int64)
    col = edge_index[1].astype(np.int64)
    E = row.shape[0]

    deg = np.bincount(row, minlength=NNODES).astype(np.float64)
    dinv = np.where(deg > 0, 1.0 / np.sqrt(np.maximum(deg, 1.0)), 0.0)
    norm = (dinv[row] * dinv[col]).astype(np.float32)

    gb = row >> 7                       # global 128-row block (0..391)
    half = (col >= SPLIT).astype(np.int64)
    order = np.lexsort((half, gb))
    gb_s, half_s, col_s, row_s, norm_s = (
        gb[order], half[order], col[order], row[order], norm[order])

    # counts per (global block, half)
    key = gb_s * 2 + half_s
    cnt = np.bincount(key, minlength=NCORES * NBLK * 2)
    cnt_lo = cnt[0::2].reshape(NCORES, NBLK)
    cnt_hi = cnt[1::2].reshape(NCORES, NBLK)
    CLO = np.maximum(1, np.ceil(cnt_lo.max(0) / 128).astype(np.int64))
    CHI = np.maximum(1, np.ceil(cnt_hi.max(0) / 128).astype(np.int64))

    LLO = int(CLO.sum()) * 128
    LHI = int(CHI.sum()) * 128
    CTOT = int(CLO.sum() + CHI.sum())

    starts = np.zeros(NCORES * NBLK * 2 + 1, np.int64)
    np.cumsum(cnt, out=starts[1:])

    per_core = []
    for c in range(NCORES):
        idx_lo = np.zeros(LLO, np.int16)
        idx_hi = np.zeros(LHI, np.int16)
        rowloc = np.zeros(CTOT * 128, np.float32)
        normv = np.zeros(CTOT * 128, np.float32)
        plo = phi = ps = 0
        for b in range(NBLK):
            k2 = (c * NBLK + b) * 2
            s0, s1 = starts[k2], starts[k2 + 1]
            nlo = s1 - s0
            idx_lo[plo:plo + nlo] = col_s[s0:s1].astype(np.int16)
            rowloc[ps:ps + nlo] = (row_s[s0:s1] & 127).astype(np.float32)
            normv[ps:ps + nlo] = norm_s[s0:s1]
            plo += CLO[b] * 128
            ps += CLO[b] * 128
            s0, s1 = starts[k2 + 1], starts[k2 + 2]
            nhi = s1 - s0
            idx_hi[phi:phi + nhi] = (col_s[s0:s1] - SPLIT).astype(np.int16)
            rowloc[ps:ps + nhi] = (row_s[s0:s1] & 127).astype(np.float32)
            normv[ps:ps + nhi] = norm_s[s0:s1]
            phi += CHI[b] * 128
            ps += CHI[b] * 128

        idx_lo_w = np.tile(idx_lo.reshape(-1, 16).T, (8, 1)).astype(np.int16)
        idx_hi_w = np.tile(idx_hi.reshape(-1, 16).T, (8, 1)).astype(np.int16)
        rowloc_w = np.ascontiguousarray(rowloc.reshape(CTOT, 128).T).astype(BF)
        normv_w = np.ascontiguousarray(normv.reshape(CTOT, 128).T).astype(BF)
        per_core.append(dict(idx_lo=idx_lo_w, idx_hi=idx_hi_w,
                             rowloc=rowloc_w, normv=normv_w))

    struct = dict(CLO=tuple(int(v) for v in CLO), CHI=tuple(int(v) for v in CHI),
                  LLO=LLO, LHI=LHI, CTOT=CTOT)
    return struct, per_core


# ----------------------------------------------------------------------------
# Bass program
# ----------------------------------------------------------------------------

def _build_program(struct):
    import concourse.bacc as bacc
    import concourse.mybir as mybir
    import concourse.tile as tile

    CLO, CHI = struct["CLO"], struct["CHI"]
    LLO, LHI, CTOT = struct["LLO"], struct["LHI"], struct["CTOT"]
    NLO_CALLS = math.ceil(LLO / GIDX)
    NHI_CALLS = math.ceil(LHI / GIDX)
    NSGRP = math.ceil(CTOT / SGRP)
    f32 = mybir.dt.float32
    bf16 = mybir.dt.bfloat16
    i16 = mybir.dt.int16

    nc = bacc.Bacc("TRN2", target_bir_lowering=False, debug=False,
                   num_devices=NCORES, dynamic_dma_scratch_size=32768)

    P = {}
    def param(name, shape, dt):
        P[name] = nc.declare_dram_parameter(name, list(shape), dt, isOutput=False)
        return P[name]

    param("idx_lo", [128, LLO // 16], i16)
    param("idx_hi", [128, LHI // 16], i16)
    param("rowloc", [128, CTOT], bf16)
    param("normv", [128, CTOT], bf16)
    param("xT", [128, RPC], bf16)
    param("w1", [128, KHOPS + 1, 128], bf16)
    param("w2", [128, KHOPS + 1, 128], bf16)
    param("b1f", [128, 128], f32)
    param("b2f", [128, 128], f32)
    param("a1c", [128, 1], f32)
    param("a2c", [128, 1], f32)
    param("wout", [128, 1], f32)
    param("boutc", [1, 1], f32)
    param("maskc", [128, NBLK], bf16)
    param("iotam", [128, 128], bf16)
    param("ident", [128, 128], f32)
    out_ext = nc.declare_dram_parameter("out", [1, 1], f32, isOutput=True)
    tablesA = [nc.declare_dram_parameter(f"tabA{i}", [SPLIT, FEAT], bf16,
                                         isOutput=True) for i in range(6)]
    tablesB = [nc.declare_dram_parameter(f"tabB{i}", [NPAD - SPLIT, FEAT], bf16,
                                         isOutput=True) for i in range(6)]

    RG = [list(range(NCORES))]

    with tile.TileContext(nc) as tc:
        with (
            tc.tile_pool(name="const", bufs=1) as cpool,
            tc.tile_pool(name="shardp", bufs=2) as shpool,
            tc.tile_pool(name="glo", bufs=8) as glop,
            tc.tile_pool(name="ghi", bufs=8) as ghip,
            tc.tile_pool(name="sgrp", bufs=4) as sgp,
            tc.tile_pool(name="work", bufs=3) as wkp,
            tc.tile_pool(name="ps", bufs=4, space="PSUM") as psp,
            tc.tile_pool(name="psx", bufs=1, space="PSUM") as psx,
            tc.tile_pool(name="dram", bufs=1, space="DRAM") as drp,
        ):
            # ---- resident constants ----
            def cload(name, shape, dt, tag):
                t = cpool.tile(shape, dt, tag=tag)
                nc.sync.dma_start(out=t[:], in_=P[name][:])
                return t

            idxlo_t = cload("idx_lo", [128, LLO // 16], i16, "idxlo")
            idxhi_t = cload("idx_hi", [128, LHI // 16], i16, "idxhi")
            rowloc_t = cload("rowloc", [128, CTOT], bf16, "rowloc")
            normv_t = cload("normv", [128, CTOT], bf16, "normv")
            xT_t = cload("xT", [128, RPC], bf16, "xT")
            w1_t = cload("w1", [128, KHOPS + 1, 128], bf16, "w1")
            w2_t = cload("w2", [128, KHOPS + 1, 128], bf16, "w2")
            b1f_t = cload("b1f", [128, 128], f32, "b1f")
            b2f_t = cload("b2f", [128, 128], f32, "b2f")
            a1c_t = cload("a1c", [128, 1], f32, "a1c")
            a2c_t = cload("a2c", [128, 1], f32, "a2c")
            wout_t = cload("wout", [128, 1], f32, "wout")
            bout_t = cload("boutc", [1, 1], f32, "bout")
            maskc_t = cload("maskc", [128, NBLK], bf16, "maskc")
            iota_t = cload("iotam", [128, 128], bf16, "iota")
            ident_t = cload("ident", [128, 128], f32, "ident")
            h1T_t = cpool.tile([128, RPC], bf16, tag="h1T")

            # ---- DRAM internals ----
            agbufs = [drp.tile([NPAD, FEAT], bf16, tag=f"agbuf{i}",
                               name=f"agbuf{i}", addr_space="Shared")
                      for i in range(6)]
            red_in = drp.tile([1, 1], f32, tag="red_in")
            red_out = drp.tile([1, 1], f32, tag="red_out", addr_space="Shared")

            def emit_gathers(ti):
                lo_bufs, hi_bufs = [], []
                for call in range(NLO_CALLS):
                    n = min(GIDX, LLO - call * GIDX)
                    gt = glop.tile([128, GIDX // 128, FEAT], bf16, tag="glo")
                    nc.gpsimd.dma_gather(
                        out_ap=gt[:, : n // 128, :],
                        in_ap=tablesA[ti][:],
                        idxs_ap=idxlo_t[:, call * (GIDX // 16):
                                        call * (GIDX // 16) + n // 16],
                        num_idxs=n, num_idxs_reg=n, elem_size=FEAT)
                    lo_bufs.append(gt)
                for call in range(NHI_CALLS):
                    n = min(GIDX, LHI - call * GIDX)
                    gt = ghip.tile([128, GIDX // 128, FEAT], bf16, tag="ghi")
                    nc.gpsimd.dma_gather(
                        out_ap=gt[:, : n // 128, :],
                        in_ap=tablesB[ti][:],
                        idxs_ap=idxhi_t[:, call * (GIDX // 16):
                                        call * (GIDX // 16) + n // 16],
                        num_idxs=n, num_idxs_reg=n, elem_size=FEAT)
                    hi_bufs.append(gt)
                return lo_bufs, hi_bufs

            def emit_sbuild():
                s_bufs = []
                for grp in range(NSGRP):
                    c0 = grp * SGRP
                    cn = min(SGRP, CTOT - c0)
                    st = sgp.tile([128, SGRP, 128], bf16, tag="sgrp")
                    nc.vector.tensor_tensor(
                        out=st[:, :cn, :],
                        in0=rowloc_t[:, c0:c0 + cn].unsqueeze(2)
                            .to_broadcast([128, cn, 128]),
                        in1=iota_t[:].unsqueeze(1).to_broadcast([128, cn, 128]),
                        op=mybir.AluOpType.is_equal)
                    nc.vector.tensor_tensor(
                        out=st[:, :cn, :],
                        in0=st[:, :cn, :],
                        in1=normv_t[:, c0:c0 + cn].unsqueeze(2)
                            .to_broadcast([128, cn, 128]),
                        op=mybir.AluOpType.mult)
                    s_bufs.append(st)
                return s_bufs

            def hop_blocks(ti, w_tile, k, lhsT_tile):
                """Yield (b, psum_tile) with the accumulated block result."""
                lo_bufs, hi_bufs = emit_gathers(ti)
                s_bufs = emit_sbuild()
                lo_sub = hi_sub = s_pos = 0
                for b in range(NBLK):
                    pt = psp.tile([128, FEAT], f32, tag="blk", space="PSUM")
                    first = True
                    for j in range(CLO[b]):
                        gs, si = lo_sub + j, s_pos + j
                        nc.tensor.matmul(
                            out=pt[:],
                            lhsT=s_bufs[si // SGRP][:, si % SGRP, :],
                            rhs=lo_bufs[gs // (GIDX // 128)][:, gs % (GIDX // 128), :],
                            start=first, stop=False)
                        first = False
                    s_pos += CLO[b]
                    for j in range(CHI[b]):
                        gs, si = hi_sub + j, s_pos + j
                        nc.tensor.matmul(
                            out=pt[:],
                            lhsT=s_bufs[si // SGRP][:, si % SGRP, :],
                            rhs=hi_bufs[gs // (GIDX // 128)][:, gs % (GIDX // 128), :],
                            start=first, stop=False)
                        first = False
                    s_pos += CHI[b]
                    lo_sub += CLO[b]
                    hi_sub += CHI[b]
                    nc.tensor.matmul(
                        out=pt[:], lhsT=lhsT_tile[:, 128 * b:128 * (b + 1)],
                        rhs=w_tile[:, k, :], start=first, stop=True)
                    yield b, pt

            def dense_blocks(w_tile, k, lhsT_tile):
                for b in range(NBLK):
                    pt = psp.tile([128, FEAT], f32, tag="blk", space="PSUM")
                    nc.tensor.matmul(
                        out=pt[:], lhsT=lhsT_tile[:, 128 * b:128 * (b + 1)],
                        rhs=w_tile[:, k, :], start=True, stop=True)
                    yield b, pt

            def store_and_gather_table(block_iter, ti):
                shard = shpool.tile([128, NBLK, FEAT], bf16, tag="shard")
                for b, pt in block_iter:
                    nc.any.tensor_copy(out=shard[:, b, :], in_=pt[:])
                bounce = drp.tile([RPC, FEAT], bf16, tag=f"bounce{ti}",
                                  name=f"bounce{ti}")
                nc.sync.dma_start(out=bounce[:], in_=shard[:])
                nc.gpsimd.collective_compute(
                    "AllGather", mybir.AluOpType.bypass, replica_groups=RG,
                    ins=[bounce.opt()], outs=[agbufs[ti].opt()])
                nc.sync.dma_start(out=tablesA[ti][:],
                                  in_=agbufs[ti][0:SPLIT, :])
                nc.sync.dma_start(out=tablesB[ti][:],
                                  in_=agbufs[ti][SPLIT:NPAD, :])

            def prelu(pt, bf_t, ac_t, dst_ap):
                biased = wkp.tile([128, 128], f32, tag="tmp1")
                neg = wkp.tile([128, 128], f32, tag="tmp2")
                nc.vector.tensor_tensor(out=biased[:], in0=pt[:], in1=bf_t[:],
                                        op=mybir.AluOpType.add)
                nc.vector.tensor_scalar(out=neg[:], in0=biased[:], scalar1=0.0,
                                        scalar2=ac_t[:, 0:1],
                                        op0=mybir.AluOpType.min,
                                        op1=mybir.AluOpType.mult)
                nc.vector.tensor_scalar(out=biased[:], in0=biased[:],
                                        scalar1=0.0, scalar2=None,
                                        op0=mybir.AluOpType.max)
                nc.vector.tensor_tensor(out=dst_ap, in0=biased[:], in1=neg[:],
                                        op=mybir.AluOpType.add)

            # ---- Layer 1, Horner ----
            # step 0: t = x @ W1[3]  -> T0
            store_and_gather_table(dense_blocks(w1_t, 3, xT_t), 0)
            # step 1: t = x @ W1[2] + A t -> T1
            if K_STEPS > 1:
                store_and_gather_table(hop_blocks(0, w1_t, 2, xT_t), 1)
            # step 2: t = x @ W1[1] + A t -> T2
            if K_STEPS > 2:
                store_and_gather_table(hop_blocks(1, w1_t, 1, xT_t), 2)
            # step 3: h1 = prelu(x @ W1[0] + A t + b1); keep h1T on chip
            if K_STEPS > 3:
                for b, pt in hop_blocks(2, w1_t, 0, xT_t):
                    h1f = wkp.tile([128, 128], f32, tag="h1f")
                    prelu(pt, b1f_t, a1c_t, h1f[:])
                    ptr = psx.tile([128, 128], f32, tag="tr", space="PSUM")
                    nc.tensor.transpose(out=ptr[:], in_=h1f[:], identity=ident_t[:])
                    nc.any.tensor_copy(out=h1T_t[:, 128 * b:128 * (b + 1)],
                                       in_=ptr[:])

            # ---- Layer 2, Horner ----
            # step 3.5: u = h1 @ W2[3] -> T3
            if K_STEPS > 4:
                store_and_gather_table(dense_blocks(w2_t, 3, h1T_t), 3)
            # step 4: u = h1 @ W2[2] + A u -> T4
            if K_STEPS > 5:
                store_and_gather_table(hop_blocks(3, w2_t, 2, h1T_t), 4)
            # step 5: u = h1 @ W2[1] + A u -> T5
            if K_STEPS > 6:
                store_and_gather_table(hop_blocks(4, w2_t, 1, h1T_t), 5)
            # step 6: h2 = prelu(h1 @ W2[0] + A u + b2); pool
            if K_STEPS > 7:
                pool_ps = psx.tile([128, 1], f32, tag="pool", space="PSUM")
                for b, pt in hop_blocks(5, w2_t, 0, h1T_t):
                    h2b = wkp.tile([128, 128], bf16, tag="h2b")
                    prelu(pt, b2f_t, a2c_t, h2b[:])
                    nc.tensor.matmul(out=pool_ps[:], lhsT=h2b[:],
                                     rhs=maskc_t[:, b:b + 1],
                                     start=(b == 0), stop=(b == NBLK - 1))

                # ---- finale: (pool @ Wout), AllReduce, + bout ----
                pv = wkp.tile([128, 1], f32, tag="pv")
                nc.any.tensor_copy(out=pv[:], in_=pool_ps[:])
                fin_ps = psx.tile([1, 1], f32, tag="fin", space="PSUM")
                nc.tensor.matmul(out=fin_ps[:], lhsT=pv[:], rhs=wout_t[:],
                                 start=True, stop=True)
                sfin = wkp.tile([1, 1], f32, tag="sfin")
                nc.any.tensor_copy(out=sfin[:], in_=fin_ps[:])
                nc.sync.dma_start(out=red_in[:], in_=sfin[:])
                nc.gpsimd.collective_compute(
                    "AllReduce", mybir.AluOpType.add, replica_groups=RG,
                    ins=[red_in.opt()], outs=[red_out.opt()])
                sred = wkp.tile([1, 1], f32, tag="sred")
                nc.sync.dma_start(out=sred[:], in_=red_out[:])
                sout = wkp.tile([1, 1], f32, tag="sout")
                nc.vector.tensor_tensor(out=sout[:], in0=sred[:], in1=bout_t[:],
                                        op=mybir.AluOpType.add)
                nc.sync.dma_start(out=out_ext[:], in_=sout[:])
            else:
                souT = wkp.tile([1, 1], f32, tag="souT")
                nc.vector.tensor_copy(out=souT[:], in_=bout_t[:])
                nc.sync.dma_start(out=out_ext[:], in_=souT[:])

    nc.finalize()
    return nc


# ----------------------------------------------------------------------------
# Per-core input maps
# ----------------------------------------------------------------------------

def _input_maps(inputs, per_core):
    x = np.asarray(inputs["x"], np.float32)
    W1 = np.asarray(inputs["W1"], np.float32)
    W2 = np.asarray(inputs["W2"], np.float32)
    b1 = np.asarray(inputs["b1"], np.float32)
    b2 = np.asarray(inputs["b2"], np.float32)
    a1 = np.asarray(inputs["a1"], np.float32)
    a2 = np.asarray(inputs["a2"], np.float32)
    Wout = np.asarray(inputs["Wout"], np.float32)
    bout = np.asarray(inputs["bout"], np.float32)

    w1s = np.ascontiguousarray(W1.transpose(1, 0, 2)).astype(BF)
    w2s = np.ascontiguousarray(W2.transpose(1, 0, 2)).astype(BF)
    b1f = np.broadcast_to(b1, (128, 128)).copy()
    b2f = np.broadcast_to(b2, (128, 128)).copy()
    a1c = np.full((128, 1), float(a1[0]), np.float32)
    a2c = np.full((128, 1), float(a2[0]), np.float32)
    woutc = Wout.reshape(128, 1).astype(np.float32)
    boutc = bout.reshape(1, 1).astype(np.float32)
    iotam = np.broadcast_to(np.arange(128, dtype=np.float32), (128, 128)).astype(BF)
    ident = np.eye(128, dtype=np.float32)

    xpad = np.zeros((NPAD, FEAT), np.float32)
    xpad[:NNODES] = x

    maps = []
    for c in range(NCORES):
        rows = np.arange(c * RPC, (c + 1) * RPC)
        xT = np.ascontiguousarray(xpad[rows].T).astype(BF)
        maskc = (rows.reshape(NBLK, 128).T < NNODES).astype(np.float32).astype(BF)
        m = dict(per_core[c])
        m.update(xT=xT, w1=w1s, w2=w2s, b1f=b1f, b2f=b2f, a1c=a1c, a2c=a2c,
                 wout=woutc, boutc=boutc, maskc=maskc, iotam=iotam, ident=ident)
        maps.append(m)
    return maps


# ----------------------------------------------------------------------------
# Entry point
# ----------------------------------------------------------------------------

def kernel(**inputs) -> np.ndarray:
    from concourse.bass_utils import run_bass_kernel_spmd

    edge_index = np.asarray(inputs["edge_index"])
    ekey = hashlib.sha1(edge_index.tobytes()).hexdigest()
    if ekey in _CACHE:
        struct, per_core, nc = _CACHE[ekey]
    else:
        struct, per_core = _preprocess(edge_index)
        nc = _build_program(struct)
        _CACHE[ekey] = (struct, per_core, nc)

    maps = _input_maps(inputs, per_core)
    res = run_bass_kernel_spmd(nc, maps, list(range(NCORES)))
    return np.asarray(res.results[0]["out"], np.float32)



# revision 14
# speedup vs baseline: 4.6682x; 4.6682x over previous
"""TAGConv×2 GNN discriminator on 8 Trainium2 NeuronCores.

Design (v2): shard nodes by SOURCE across 8 cores. Each hop gathers
per-edge source-feature rows from the core's OWN 1.6MB table (local DRAM,
int16 indices), scatter-adds them into per-destination-block PSUM tiles via
binary one-hot S matmuls, and combines partial destination sums across
cores with a single ReduceScatter per hop (output = own 6272-row shard,
~4.6x cheaper than the old full-table AllGather + dram2dram copy).

The symmetric norm deg^-1/2 A deg^-1/2 is factored: tables store
u = d .* t (pre-scaled by source dinv), S is pure 0/1 (built in ONE DVE
is_equal pass; dead slots get rowloc=255), and the destination dinv is
applied post-ReduceScatter, fused with the dense Horner term via one
scalar_tensor_tensor op per 128-row block. PReLU runs on the Activation
engine (Lrelu, alpha AP). Bias enters via a tiny rank-1 matmul into the
same PSUM as the dense term.

Gather streams are densely packed per destination block with
uniform-across-cores slot counts (SLOT[b] = max_c cnt[c,b]); chunks that
straddle a block boundary get two S columns (one per block) so the SPMD
program is identical on every core while padding stays ~13%.
"""
import hashlib
import math

import numpy as np
import ml_dtypes

BF = ml_dtypes.bfloat16

NCORES = 8
NNODES = 50000
FEAT = 128
KHOPS = 3
NBLK = 49                 # 128-row blocks per core
RPC = NBLK * 128          # rows per core (6272)
NPAD = NCORES * RPC       # padded node count (50176)
NGBLK = NCORES * NBLK     # global dest blocks (392)
GIDX = 1024               # indices per dma_gather call
SGRP = 16                 # S columns per DVE is_equal op

_CACHE: dict = {}


# ----------------------------------------------------------------------------
# Host preprocessing: graph -> per-core gather streams + program structure
# ----------------------------------------------------------------------------

def _preprocess(edge_index: np.ndarray):
    row = edge_index[0].astype(np.int64)   # dst
    col = edge_index[1].astype(np.int64)   # src
    E = row.shape[0]

    deg = np.bincount(row, minlength=NNODES).astype(np.float64)
    dinv_full = np.where(deg > 0, 1.0 / np.sqrt(np.maximum(deg, 1.0)), 0.0)
    dinv_pad = np.zeros(NPAD, np.float64)
    dinv_pad[:NNODES] = dinv_full

    src_core = col // RPC                 # owning core of the edge's source
    gb = row >> 7                         # global dest block (0..391)

    # per (core, block) counts -> uniform slot allocation
    key = src_core * NGBLK + gb
    cnt = np.bincount(key, minlength=NCORES * NGBLK).reshape(NCORES, NGBLK)
    SLOT = cnt.max(axis=0)                # uniform across cores
    slot_start = np.zeros(NGBLK + 1, np.int64)
    np.cumsum(SLOT, out=slot_start[1:])
    L = int(slot_start[-1])
    L128 = (L + 127) & ~127
    NCHUNK = L128 // 128

    # (chunk, block) pair list — identical for every core
    pairs = []            # (block, s_col) in stream order per chunk
    chunk_pairs = []      # per chunk: list of (pair_idx, block)
    for k in range(NCHUNK):
        lo, hi = 128 * k, 128 * (k + 1)
        b0 = int(np.searchsorted(slot_start, lo, side="right")) - 1
        cps = []
        b = max(b0, 0)
        while b < NGBLK and slot_start[b] < hi:
            if slot_start[b + 1] > lo and SLOT[b] > 0:
                cps.append((len(pairs), b))
                pairs.append(b)
            b += 1
        chunk_pairs.append(cps)
    CTOT = len(pairs)

    # per-core streams
    per_core = []
    order = np.lexsort((row, gb))   # by dest block (then dest row for locality)
    for c in range(NCORES):
        m = src_core[order] == c
        oc = order[m]
        gb_c = gb[oc]
        # position of each edge within its block's slot range
        idx_stream = np.zeros(L128, np.int16)
        rloc = np.full((CTOT, 128), 255.0, np.float32)
        # offsets: edges sorted by gb; place consecutively from slot_start[b]
        blk_cnt = np.bincount(gb_c, minlength=NGBLK)
        pos_in_blk = np.arange(len(oc)) - np.repeat(
            np.concatenate([[0], np.cumsum(blk_cnt)[:-1]]), blk_cnt)
        slot = slot_start[gb_c] + pos_in_blk
        idx_stream[slot] = (col[oc] - c * RPC).astype(np.int16)
        # S columns: for pair (p, b): edges in chunk with that block
        dstrow = (row[oc] & 127).astype(np.float32)
        chunk_of_slot = slot // 128
        lane = slot % 128
        pair_lookup = {}
        for k, cps in enumerate(chunk_pairs):
            for p, b in cps:
                pair_lookup[(k, b)] = p
        p_of_edge = np.fromiter(
            (pair_lookup[(int(k), int(b))] for k, b in zip(chunk_of_slot, gb_c)),
            np.int64, len(oc))
        rloc[p_of_edge, lane] = dstrow

        idx_w = np.tile(idx_stream.reshape(-1, 16).T, (8, 1)).astype(np.int16)
        rowloc_w = np.ascontiguousarray(rloc.T).astype(BF)   # [128, CTOT]
        dv = dinv_pad[c * RPC:(c + 1) * RPC]
        d1 = np.ascontiguousarray(
            dv.reshape(NBLK, 128).T).astype(np.float32)      # [128, NBLK]
        per_core.append(dict(idx=idx_w, rowloc=rowloc_w, d1=d1,
                             dsq=(d1 * d1).astype(np.float32),
                             dinv=dv.copy()))

    struct = dict(L128=L128, NCHUNK=NCHUNK, CTOT=CTOT,
                  chunk_pairs=tuple(tuple(cp) for cp in chunk_pairs))
    return struct, per_core


# ----------------------------------------------------------------------------
# Bass program
# ----------------------------------------------------------------------------

def _build_program(struct):
    import concourse.bacc as bacc
    import concourse.mybir as mybir
    import concourse.tile as tile

    L128 = struct["L128"]
    NCHUNK = struct["NCHUNK"]
    CTOT = struct["CTOT"]
    chunk_pairs = struct["chunk_pairs"]
    NCALL = math.ceil(L128 / GIDX)
    NSG = math.ceil(CTOT / SGRP)
    f32 = mybir.dt.float32
    bf16 = mybir.dt.bfloat16
    i16 = mybir.dt.int16

    nc = bacc.Bacc("TRN2", target_bir_lowering=False, debug=False,
                   num_devices=NCORES, dynamic_dma_scratch_size=32768)

    P = {}
    def param(name, shape, dt):
        P[name] = nc.declare_dram_parameter(name, list(shape), dt, isOutput=False)
        return P[name]

    param("idx", [128, L128 // 16], i16)
    param("rowloc", [128, CTOT], bf16)
    param("iotam", [128, 128], bf16)
    param("xT", [128, RPC], bf16)
    param("xdT", [128, RPC], bf16)
    param("w1", [128, KHOPS + 1, 128], bf16)
    param("w2", [128, KHOPS + 1, 128], bf16)
    param("bias1", [128, 128], bf16)
    param("bias2", [128, 128], bf16)
    param("i128", [128, 128], bf16)
    param("na1c", [128, 1], f32)
    param("na2c", [128, 1], f32)
    param("d1", [128, NBLK], f32)
    param("dsq", [128, NBLK], f32)
    param("wout", [128, 1], f32)
    param("boutc", [1, 1], f32)
    param("maskc", [128, NBLK], bf16)
    param("ident", [128, 128], f32)
    out_ext = nc.declare_dram_parameter("out", [1, 1], f32, isOutput=True)
    tables = [nc.declare_dram_parameter(f"tab{i}", [RPC, FEAT], bf16,
                                        isOutput=True) for i in range(6)]

    RG = [list(range(NCORES))]

    with tile.TileContext(nc) as tc:
        with (
            tc.tile_pool(name="const", bufs=1) as cpool,
            tc.tile_pool(name="gath", bufs=3) as gpool,
            tc.tile_pool(name="sgrp", bufs=3) as spool,
            tc.tile_pool(name="ystag", bufs=2) as ypool,
            tc.tile_pool(name="ustag", bufs=2) as upool,
            tc.tile_pool(name="yin", bufs=2) as yinp,
            tc.tile_pool(name="work", bufs=4) as wkp,
            tc.tile_pool(name="ps", bufs=3, space="PSUM") as psp,
            tc.tile_pool(name="psd", bufs=2, space="PSUM") as psd,
            tc.tile_pool(name="psx", bufs=1, space="PSUM") as psx,
            tc.tile_pool(name="dram", bufs=1, space="DRAM") as drp,
        ):
            def cload(name, shape, dt, tag):
                t = cpool.tile(shape, dt, tag=tag)
                nc.sync.dma_start(out=t[:], in_=P[name][:])
                return t

            idx_t = cload("idx", [128, L128 // 16], i16, "idx")
            rowloc_t = cload("rowloc", [128, CTOT], bf16, "rowloc")
            iota_t = cload("iotam", [128, 128], bf16, "iota")
            xT_t = cload("xT", [128, RPC], bf16, "xT")
            xdT_t = cload("xdT", [128, RPC], bf16, "xdT")
            w1_t = cload("w1", [128, KHOPS + 1, 128], bf16, "w1")
            w2_t = cload("w2", [128, KHOPS + 1, 128], bf16, "w2")
            bias1_t = cload("bias1", [128, 128], bf16, "bias1")
            bias2_t = cload("bias2", [128, 128], bf16, "bias2")
            i128_t = cload("i128", [128, 128], bf16, "i128")
            na1c_t = cload("na1c", [128, 1], f32, "na1c")
            na2c_t = cload("na2c", [128, 1], f32, "na2c")
            d1_t = cload("d1", [128, NBLK], f32, "d1")
            dsq_t = cload("dsq", [128, NBLK], f32, "dsq")
            wout_t = cload("wout", [128, 1], f32, "wout")
            bout_t = cload("boutc", [1, 1], f32, "bout")
            maskc_t = cload("maskc", [128, NBLK], bf16, "maskc")
            ident_t = cload("ident", [128, 128], f32, "ident")
            h1T_t = cpool.tile([128, RPC], bf16, tag="h1T")
            h1dT_t = cpool.tile([128, RPC], bf16, tag="h1dT")

            def emit_gathers(ti):
                bufs = []
                for call in range(NCALL):
                    n = min(GIDX, L128 - call * GIDX)
                    gt = gpool.tile([128, GIDX // 128, FEAT], bf16, tag="gt")
                    nc.gpsimd.dma_gather(
                        out_ap=gt[:, : n // 128, :],
                        in_ap=tables[ti][:],
                        idxs_ap=idx_t[:, call * (GIDX // 16):
                                      call * (GIDX // 16) + n // 16],
                        num_idxs=n, num_idxs_reg=n, elem_size=FEAT)
                    bufs.append(gt)
                return bufs

            def emit_sbuild():
                s_bufs = []
                for g in range(NSG):
                    c0 = g * SGRP
                    cn = min(SGRP, CTOT - c0)
                    st = spool.tile([128, SGRP, 128], bf16, tag="st")
                    nc.vector.tensor_tensor(
                        out=st[:, :cn, :],
                        in0=rowloc_t[:, c0:c0 + cn].unsqueeze(2)
                            .to_broadcast([128, cn, 128]),
                        in1=iota_t[:].unsqueeze(1)
                            .to_broadcast([128, cn, 128]),
                        op=mybir.AluOpType.is_equal)
                    s_bufs.append(st)
                return s_bufs

            block_mms: dict = {}
            for k in range(NCHUNK):
                for p, b in chunk_pairs[k]:
                    block_mms.setdefault(b, []).append((p, k))
            blocks_order = sorted(block_mms)

            def scatter_blocks(ti):
                """Yield (b, psum) per global dest block, in order."""
                g_bufs = emit_gathers(ti)
                s_bufs = emit_sbuild()
                for b in blocks_order:
                    lst = block_mms[b]
                    pt = psp.tile([128, FEAT], f32, tag="blk", space="PSUM")
                    for i, (p, k) in enumerate(lst):
                        gt = g_bufs[(128 * k) // GIDX]
                        gcol = (128 * k) % GIDX // 128
                        st = s_bufs[p // SGRP]
                        nc.tensor.matmul(
                            out=pt[:],
                            lhsT=st[:, p % SGRP, :],
                            rhs=gt[:, gcol, :],
                            start=(i == 0), stop=(i == len(lst) - 1))
                    yield b, pt

            def hop_partials(ti_src, ti_y):
                """Scatter + per-owner staging + y write + ReduceScatter.
                Returns the [128, NBLK, 128] bf16 SBUF tile of reduced
                partial sums for this core's shard."""
                y_dram = drp.tile([NCORES, 128, NBLK, FEAT], bf16,
                                  tag=f"y{ti_y}", name=f"y{ti_y}")
                ysh = drp.tile([128, NBLK, FEAT], bf16, tag=f"ysh{ti_y}",
                               name=f"ysh{ti_y}")
                state = {"stag": None, "o": -1}

                def slot_ap(bb):
                    o, j = bb // NBLK, bb % NBLK
                    if o != state["o"]:
                        if state["stag"] is not None:
                            nc.sync.dma_start(out=y_dram[state["o"]],
                                              in_=state["stag"][:])
                        stag = ypool.tile([128, NBLK, FEAT], bf16,
                                          tag="ystag", name="ystag")
                        state["stag"] = stag
                        state["o"] = o
                    return state["stag"][:, j, :]

                next_b = 0
                for b, pt in scatter_blocks(ti_src):
                    for bb in range(next_b, b):
                        nc.any.memset(slot_ap(bb), 0.0)
                    nc.any.tensor_copy(out=slot_ap(b), in_=pt[:])
                    next_b = b + 1
                for bb in range(next_b, NGBLK):
                    nc.any.memset(slot_ap(bb), 0.0)
                nc.sync.dma_start(out=y_dram[state["o"]], in_=state["stag"][:])
                nc.gpsimd.collective_compute(
                    "ReduceScatter", mybir.AluOpType.add, replica_groups=RG,
                    ins=[y_dram.opt()], outs=[ysh.opt()])
                yt = yinp.tile([128, NBLK, FEAT], bf16, tag="yt")
                nc.sync.dma_start(out=yt[:], in_=ysh[:])
                return yt

            def dense_psum(lhsT_t, w_t, k, b, bias_t=None):
                pt = psd.tile([128, FEAT], f32, tag="dblk", space="PSUM")
                nc.tensor.matmul(
                    out=pt[:], lhsT=lhsT_t[:, 128 * b:128 * (b + 1)],
                    rhs=w_t[:, k, :], start=True, stop=bias_t is None)
                if bias_t is not None:
                    nc.tensor.matmul(out=pt[:], lhsT=i128_t[:],
                                     rhs=bias_t[:], start=False, stop=True)
                return pt

            def write_table(u_iter, ti):
                """u_iter yields (b, bf16_ap_filled_into_staging)."""
                stag = upool.tile([128, NBLK, FEAT], bf16, tag="ustag")
                for b in u_iter(stag):
                    pass
                nc.sync.dma_start(out=tables[ti][:], in_=stag[:])

            # ---- layer runner ----
            def layer(xT_l, xdT_l, w_t, bias_t, ac_t, t_first, is_l2):
                # step A: u = (d .* x) @ W[3]  -> table t_first
                def densefill(stag):
                    for b in range(NBLK):
                        pt = dense_psum(xdT_l, w_t, KHOPS, b)
                        nc.any.tensor_copy(out=stag[:, b, :], in_=pt[:])
                        yield b
                write_table(densefill, t_first)

                # steps B,C: mid hops
                for hop in range(1, KHOPS):
                    ti_src = t_first + hop - 1
                    ti_dst = t_first + hop
                    k = KHOPS - hop
                    yt = hop_partials(ti_src, ti_src)

                    def midfill(stag, yt=yt, k=k):
                        for b in range(NBLK):
                            pt = dense_psum(xdT_l, w_t, k, b)
                            nc.vector.scalar_tensor_tensor(
                                out=stag[:, b, :],
                                in0=yt[:, b, :],
                                scalar=dsq_t[:, b:b + 1],
                                in1=pt[:],
                                op0=mybir.AluOpType.mult,
                                op1=mybir.AluOpType.add)
                            yield b
                    write_table(midfill, ti_dst)

                # final hop: t = x @ W[0] + bias + d .* y ; h = prelu(t)
                ti_src = t_first + KHOPS - 1
                yt = hop_partials(ti_src, ti_src)
                for b in range(NBLK):
                    pt = dense_psum(xT_l, w_t, 0, b, bias_t=bias_t)
                    tf = wkp.tile([128, 128], f32, tag="tf")
                    nc.vector.scalar_tensor_tensor(
                        out=tf[:], in0=yt[:, b, :], scalar=d1_t[:, b:b + 1],
                        in1=pt[:], op0=mybir.AluOpType.mult,
                        op1=mybir.AluOpType.add)
                    yield b, tf

            # ---- Layer 1 ----
            ustag1 = cpool.tile([128, NBLK, FEAT], bf16, tag="ustag1")
            for b, tf in layer(xT_t, xdT_t, w1_t, bias1_t, na1c_t, 0, False):
                hp = wkp.tile([128, 128], f32, tag="hp")
                nc.scalar.activation(out=hp[:], in_=tf[:],
                                     func=mybir.ActivationFunctionType.Relu)
                hn = wkp.tile([128, 128], f32, tag="hn")
                nc.scalar.activation(out=hn[:], in_=tf[:], scale=-1.0,
                                     func=mybir.ActivationFunctionType.Relu)
                h1f = wkp.tile([128, 128], f32, tag="h1f")
                nc.vector.scalar_tensor_tensor(
                    out=h1f[:], in0=hn[:], scalar=na1c_t[:, 0:1], in1=hp[:],
                    op0=mybir.AluOpType.mult, op1=mybir.AluOpType.add)
                h1df = wkp.tile([128, 128], f32, tag="h1df")
                nc.vector.tensor_scalar(out=h1df[:], in0=h1f[:],
                                        scalar1=d1_t[:, b:b + 1], scalar2=None,
                                        op0=mybir.AluOpType.mult)
                nc.any.tensor_copy(out=ustag1[:, b, :], in_=h1df[:])
                ptr = psx.tile([128, 128], f32, tag="tr", space="PSUM")
                nc.tensor.transpose(out=ptr[:], in_=h1f[:], identity=ident_t[:])
                nc.any.tensor_copy(out=h1T_t[:, 128 * b:128 * (b + 1)],
                                   in_=ptr[:])
                ptr2 = psx.tile([128, 128], f32, tag="tr", space="PSUM")
                nc.tensor.transpose(out=ptr2[:], in_=h1df[:], identity=ident_t[:])
                nc.any.tensor_copy(out=h1dT_t[:, 128 * b:128 * (b + 1)],
                                   in_=ptr2[:])
            nc.sync.dma_start(out=tables[3][:], in_=ustag1[:])

            # ---- Layer 2 ----
            pool_ps = psx.tile([128, 1], f32, tag="pool", space="PSUM")
            for b, tf in layer(h1T_t, h1dT_t, w2_t, bias2_t, na2c_t, 3, True):
                hp2 = wkp.tile([128, 128], f32, tag="hp")
                nc.scalar.activation(out=hp2[:], in_=tf[:],
                                     func=mybir.ActivationFunctionType.Relu)
                hn2 = wkp.tile([128, 128], f32, tag="hn")
                nc.scalar.activation(out=hn2[:], in_=tf[:], scale=-1.0,
                                     func=mybir.ActivationFunctionType.Relu)
                h2b = wkp.tile([128, 128], bf16, tag="h2b")
                nc.vector.scalar_tensor_tensor(
                    out=h2b[:], in0=hn2[:], scalar=na2c_t[:, 0:1], in1=hp2[:],
                    op0=mybir.AluOpType.mult, op1=mybir.AluOpType.add)
                nc.tensor.matmul(out=pool_ps[:], lhsT=h2b[:],
                                 rhs=maskc_t[:, b:b + 1],
                                 start=(b == 0), stop=(b == NBLK - 1))

            # ---- finale ----
            red_in = drp.tile([1, 1], f32, tag="red_in")
            red_out = drp.tile([1, 1], f32, tag="red_out", addr_space="Shared")
            pv = wkp.tile([128, 1], f32, tag="pv")
            nc.any.tensor_copy(out=pv[:], in_=pool_ps[:])
            fin_ps = psx.tile([1, 1], f32, tag="fin", space="PSUM")
            nc.tensor.matmul(out=fin_ps[:], lhsT=pv[:], rhs=wout_t[:],
                             start=True, stop=True)
            sfin = wkp.tile([1, 1], f32, tag="sfin")
            nc.any.tensor_copy(out=sfin[:], in_=fin_ps[:])
            nc.sync.dma_start(out=red_in[:], in_=sfin[:])
            nc.gpsimd.collective_compute(
                "AllReduce", mybir.AluOpType.add, replica_groups=RG,
                ins=[red_in.opt()], outs=[red_out.opt()])
            sred = wkp.tile([1, 1], f32, tag="sred")
            nc.sync.dma_start(out=sred[:], in_=red_out[:])
            sout = wkp.tile([1, 1], f32, tag="sout")
            nc.vector.tensor_tensor(out=sout[:], in0=sred[:], in1=bout_t[:],
                                    op=mybir.AluOpType.add)
            nc.sync.dma_start(out=out_ext[:], in_=sout[:])

    nc.finalize()
    return nc


# ----------------------------------------------------------------------------
# Per-core input maps
# ----------------------------------------------------------------------------

def _input_maps(inputs, per_core):
    x = np.asarray(inputs["x"], np.float32)
    W1 = np.asarray(inputs["W1"], np.float32)
    W2 = np.asarray(inputs["W2"], np.float32)
    b1 = np.asarray(inputs["b1"], np.float32)
    b2 = np.asarray(inputs["b2"], np.float32)
    a1 = np.asarray(inputs["a1"], np.float32)
    a2 = np.asarray(inputs["a2"], np.float32)
    Wout = np.asarray(inputs["Wout"], np.float32)
    bout = np.asarray(inputs["bout"], np.float32)

    w1s = np.ascontiguousarray(W1.transpose(1, 0, 2)).astype(BF)
    w2s = np.ascontiguousarray(W2.transpose(1, 0, 2)).astype(BF)
    bias1 = np.broadcast_to(b1, (128, 128)).astype(BF).copy()
    bias2 = np.broadcast_to(b2, (128, 128)).astype(BF).copy()
    i128 = np.full((128, 128), 1.0 / 128.0, BF)
    na1c = np.full((128, 1), -float(a1[0]), np.float32)
    na2c = np.full((128, 1), -float(a2[0]), np.float32)
    woutc = Wout.reshape(128, 1).astype(np.float32)
    boutc = bout.reshape(1, 1).astype(np.float32)
    iotam = np.broadcast_to(np.arange(128, dtype=np.float32),
                            (128, 128)).astype(BF).copy()
    ident = np.eye(128, dtype=np.float32)

    xpad = np.zeros((NPAD, FEAT), np.float32)
    xpad[:NNODES] = x

    maps = []
    for c in range(NCORES):
        rows = np.arange(c * RPC, (c + 1) * RPC)
        xc = xpad[rows]
        xT = np.ascontiguousarray(xc.T).astype(BF)
        xdT = np.ascontiguousarray((xc * per_core[c]["dinv"][:, None]).T
                                   ).astype(BF)
        maskc = (rows.reshape(NBLK, 128).T < NNODES).astype(np.float32
                                                            ).astype(BF)
        m = dict(idx=per_core[c]["idx"], rowloc=per_core[c]["rowloc"],
                 d1=per_core[c]["d1"], dsq=per_core[c]["dsq"])
        m.update(xT=xT, xdT=xdT, w1=w1s, w2=w2s, bias1=bias1, bias2=bias2,
                 i128=i128, na1c=na1c, na2c=na2c, wout=woutc, boutc=boutc,
                 maskc=maskc, iotam=iotam, ident=ident)
        maps.append(m)
    return maps


# ----------------------------------------------------------------------------
# Entry point
# ----------------------------------------------------------------------------

def kernel(**inputs) -> np.ndarray:
    from concourse.bass_utils import run_bass_kernel_spmd

    edge_index = np.asarray(inputs["edge_index"])
    ekey = hashlib.sha1(edge_index.tobytes()).hexdigest()
    if ekey in _CACHE:
        struct, per_core, nc = _CACHE[ekey]
    else:
        struct, per_core = _preprocess(edge_index)
        nc = _build_program(struct)
        _CACHE[ekey] = (struct, per_core, nc)

    maps = _input_maps(inputs, per_core)
    res = run_bass_kernel_spmd(nc, maps, list(range(NCORES)))
    return np.asarray(res.results[0]["out"], np.float32)
